# revision 4
# baseline (speedup 1.0000x reference)
"""MoE LoRA linear kernel for Trainium2, 8 NeuronCores, data-parallel over tokens.

Reference computation (per token x, D=4096, E=28 experts, rank 8, top-2):
  base   = x @ W^T
  logits = x @ gate_W^T ; top-2 softmax -> per-expert gates g (0 elsewhere)
  h_e    = x @ A_e^T                     (all experts, rank 8)
  out    = base + sum_e g_e*2 * h_e @ B_e^T

Sharding: tokens split 8 ways (1024 tokens/core); weights replicated.

Numerics: everything bf16 on the PE (fp32 PSUM accumulation); inputs cast to
bf16 on host.  bf16 keeps the PE at 1 cycle/row and enables FWL (fast weight
load), which fp32/fp32r weights cannot use, halving the LDWEIGHTS cost of the
1024+ stationary loads in the base GEMM.  Output stored bf16, upcast on host.
End-to-end rel err ~2e-3 vs the 2e-2 gate.

Structure (per core):
  - x DMAs land directly in the resident x tile (no DVE cast): layout
    [P, TT, KT, 128] so each token tile is one contiguous 2MB DMA on the
    scalar-engine HWDGE queue (W uses the sync-engine queue).
  - phase-1 per token tile: 32 matmuls vs the combined [A^T | gate_W^T]
    (224 lora cols + 28 gate cols in one 256-wide PSUM accumulation), PSUM
    copied to bf16 h_all + f32 logits; batched top-2 softmax; gate-scale
    multiply; PE transpose -> h'T halves (hta/htb).
  - main GEMM in 13 passes over j-groups of 3 (2 PSUM banks x 512-token
    chunk): passes 0-1 run token-chunk 0 only and park their W tiles in SBUF
    (wg_sb); passes 2-10 run both chunks streaming W in 4-ktile chunks;
    passes 11-12 run the deferred chunk 1 of j 0-5 from the parked W tiles.
    Token tiles 4-7 phase-1 work is hooked inside pass 1's k-loop.
  - each pass ends with the 2 lora rank-combine matmuls per (j, chunk)
    accumulated into the same PSUM, bf16 copy-out, store.
"""
import sys

for _p in ("/opt/trn_rl_repo", "/root/.axon_site/_ro/trn_rl_repo"):
    if _p not in sys.path:
        sys.path.insert(0, _p)

import numpy as np
import ml_dtypes

import concourse.bass as bass
import concourse.mybir as mybir
import concourse.tile as tile
from concourse import bacc, bass_utils
from concourse.masks import make_identity

F32 = mybir.dt.float32
BF16 = mybir.dt.bfloat16
NPBF16 = ml_dtypes.bfloat16

N_CORES = 8
B, S, D_IN, D_OUT = 4, 2048, 4096, 4096
N_EXPERTS, RANK, SCALING = 28, 8, 2.0
ER = N_EXPERTS * RANK          # 224
T = B * S // N_CORES           # 1024 tokens per core
P = 128
KT = D_IN // P                 # 32 k-tiles
JT = D_OUT // P                # 32 output row-tiles
TT = T // P                    # 8 token tiles
NCH = 512                      # moving free dim chunk (1 PSUM bank fp32)
JG = 3                         # j-tiles per psum group
WQ = 4                         # k-tiles per W DMA
AT_COLS = 256                  # 224 lora + 28 gate + 4 pad
NKEEP = 2                      # leading j-groups whose W stays resident


def build_nc(repeat=1):
    nc = bacc.Bacc("TRN2", target_bir_lowering=False, debug=False)
    xg_d = nc.dram_tensor("xg", [TT, P, KT * P], BF16,
                          kind="ExternalInput").ap()
    wt_d = nc.dram_tensor("wt", [D_IN, D_OUT], BF16, kind="ExternalInput").ap()
    ga_d = nc.dram_tensor("ga", [P, KT * AT_COLS], BF16,
                          kind="ExternalInput").ap()
    bab_d = nc.dram_tensor("bab", [2 * P, D_OUT], BF16,
                           kind="ExternalInput").ap()
    out_d = nc.dram_tensor("out", [D_OUT, T], BF16, kind="ExternalOutput").ap()

    ga_re = ga_d.rearrange("p (kt c) -> p kt c", kt=KT)
    wt_re = wt_d.rearrange("(kt p) j -> p kt j", p=P)
    bab_re = bab_d.rearrange("(i p) j -> p i j", p=P)

    # passes: (j0, nj, chunks, keep_idx)
    #   keep_idx >= 0 on the first NKEEP passes (W parked in wg_sb) and on
    #   their deferred chunk-1 passes at the end (W read back, no DMA).
    passes = [(g * JG, JG, (0,), g) for g in range(NKEEP)]
    ns = NKEEP * JG
    nfull = (JT - ns) // JG
    passes += [(ns + g * JG, JG, (0, 1), -1) for g in range(nfull)]
    if (JT - ns) % JG:
        passes.append((ns + nfull * JG, (JT - ns) % JG, (0, 1), -1))
    passes += [(g * JG, JG, (1,), g) for g in range(NKEEP)]

    with tile.TileContext(nc) as tc:
        with (
            tc.tile_pool(name="resident", bufs=1) as rp,
            tc.tile_pool(name="wstream", bufs=4) as wp,
            tc.tile_pool(name="bstream", bufs=2) as bp,
            tc.tile_pool(name="outstage", bufs=4) as op_,
            tc.tile_pool(name="smalls", bufs=2) as sp,
            tc.tile_pool(name="gating", bufs=1) as gp,
            tc.tile_pool(name="ph1ps", bufs=2, space="PSUM") as ph1,
            tc.tile_pool(name="psmm", bufs=6, space="PSUM") as psm,
        ):
            for _rep in range(repeat):
                ga_sb = rp.tile([P, KT, AT_COLS], BF16)
                nc.sync.dma_start(ga_sb[:], ga_re[:])
                ident = rp.tile([P, P], BF16)
                make_identity(nc, ident[:])
                xt_sb = rp.tile([P, TT, KT, P], BF16)
                wg_sb = [rp.tile([P, KT, JG * P], BF16, name=f"wg{g}")
                         for g in range(NKEEP)]
                h_all = rp.tile([P, TT, AT_COLS], BF16)
                hta_sb = rp.tile([P, T], BF16)
                htb_sb = rp.tile([P, T], BF16)
                logits_all = rp.tile([P, TT, N_EXPERTS], F32)
                gsc_all = rp.tile([P, TT, N_EXPERTS], F32)

                for t in range(TT):
                    nc.scalar.dma_start(
                        xt_sb[:, t],
                        xg_d[t].rearrange("p (kt f) -> p kt f", kt=KT))

                def tile_mms(t):
                    """32 matmuls vs [A^T | gate_W^T]; stash h (bf16) and
                    logits (f32)."""
                    ph = ph1.tile([P, AT_COLS], F32, name="ph", tag="ph1")
                    for kt in range(KT):
                        nc.tensor.matmul(ph[:], xt_sb[:, t, kt], ga_sb[:, kt],
                                         start=(kt == 0), stop=(kt == KT - 1))
                    nc.vector.tensor_copy(logits_all[:, t],
                                          ph[:, ER:ER + N_EXPERTS])
                    nc.vector.tensor_copy(h_all[:, t], ph[:])

                def gate_chain(lo, hi):
                    """Batched top-2 softmax (x SCALING) for tiles [lo, hi)."""
                    n = hi - lo
                    sl = slice(lo, hi)
                    EB = (P, n, N_EXPERTS)
                    m1 = gp.tile([P, n], F32, name=f"m1_{lo}", tag="m1")
                    nc.vector.reduce_max(m1[:], logits_all[:, sl],
                                         axis=mybir.AxisListType.X)
                    m1b = m1[:, :, None].to_broadcast(EB)
                    eq = gp.tile([P, n, N_EXPERTS], F32, name=f"eq_{lo}",
                                 tag="eq")
                    nc.vector.tensor_tensor(eq[:], logits_all[:, sl], m1b,
                                            mybir.AluOpType.is_equal)
                    nc.vector.scalar_tensor_tensor(
                        eq[:], eq[:], -1e30, logits_all[:, sl],
                        mybir.AluOpType.mult, mybir.AluOpType.add)
                    m2 = gp.tile([P, n], F32, name=f"m2_{lo}", tag="m2")
                    nc.vector.reduce_max(m2[:], eq[:],
                                         axis=mybir.AxisListType.X)
                    mask2 = gp.tile([P, n, N_EXPERTS], F32, name=f"mask2_{lo}",
                                    tag="mask2")
                    nc.vector.tensor_tensor(mask2[:], logits_all[:, sl],
                                            m2[:, :, None].to_broadcast(EB),
                                            mybir.AluOpType.is_ge)
                    d1 = gp.tile([P, n, N_EXPERTS], F32, name=f"d1_{lo}",
                                 tag="d1")
                    nc.vector.tensor_tensor(d1[:], logits_all[:, sl], m1b,
                                            mybir.AluOpType.subtract)
                    nc.scalar.activation(d1[:], d1[:],
                                         mybir.ActivationFunctionType.Exp)
                    d2 = gp.tile([P, n], F32, name=f"d2_{lo}", tag="d2")
                    nc.vector.tensor_tensor(d2[:], m2[:], m1[:],
                                            mybir.AluOpType.subtract)
                    nc.scalar.activation(d2[:], d2[:],
                                         mybir.ActivationFunctionType.Exp)
                    nc.vector.tensor_scalar_add(d2[:], d2[:], 1.0)
                    nc.vector.reciprocal(d2[:], d2[:])
                    nc.vector.tensor_scalar_mul(d2[:], d2[:], SCALING)
                    nc.vector.tensor_tensor(d1[:], d1[:], mask2[:],
                                            mybir.AluOpType.mult)
                    nc.vector.tensor_tensor(gsc_all[:, sl], d1[:],
                                            d2[:, :, None].to_broadcast(EB),
                                            mybir.AluOpType.mult)

                def scale_transpose(t):
                    """h * gate-scale (bf16) then PE-transpose into h'T."""
                    ts_ = slice(t * P, (t + 1) * P)
                    hp = sp.tile([P, AT_COLS], BF16, name="hp")
                    nc.vector.memset(hp[:, ER:], 0.0)
                    nc.vector.tensor_tensor(
                        hp[:, :ER].rearrange("p (e r) -> p e r", r=RANK),
                        h_all[:, t, :ER].rearrange("p (e r) -> p e r", r=RANK),
                        gsc_all[:, t, :, None].to_broadcast(
                            (P, N_EXPERTS, RANK)),
                        mybir.AluOpType.mult)
                    for half, dst in ((0, hta_sb), (1, htb_sb)):
                        pt = ph1.tile([P, P], BF16, name="pt", tag="ph1")
                        nc.tensor.transpose(
                            pt[:], hp[:, half * P:(half + 1) * P], ident[:])
                        nc.vector.tensor_copy(dst[:, ts_], pt[:])

                # ---- prologue: token tiles 0-3 ----
                for t in range(TT // 2):
                    tile_mms(t)
                gate_chain(0, TT // 2)
                for t in range(TT // 2):
                    scale_transpose(t)

                # phase-1 work for tiles 4-7 hooked into pass 1's k-loop
                hooks = {
                    4: lambda: tile_mms(4),
                    10: lambda: tile_mms(5),
                    16: lambda: tile_mms(6),
                    22: lambda: tile_mms(7),
                    26: lambda: gate_chain(TT // 2, TT),
                    28: lambda: scale_transpose(4),
                    29: lambda: scale_transpose(5),
                    30: lambda: scale_transpose(6),
                    31: lambda: scale_transpose(7),
                }

                b_tiles = {}

                def load_b(pi):
                    if pi >= len(passes):
                        return
                    j0, nj, _, _ = passes[pi]
                    js = slice(j0 * P, (j0 + nj) * P)
                    bt = bp.tile([P, 2, JG * P], BF16, name="bt")
                    nc.sync.dma_start(bt[:, :, :nj * P], bab_re[:, :, js])
                    b_tiles[pi] = bt

                load_b(0)
                for pi, (j0, nj, chunks, keep) in enumerate(passes):
                    js = slice(j0 * P, (j0 + nj) * P)
                    deferred = keep >= 0 and pi >= len(passes) - NKEEP
                    psums = {
                        (j, c): psm.tile([P, NCH], F32, name=f"pm_{j}_{c}",
                                         tag="pm")
                        for j in range(nj) for c in chunks
                    }
                    for kq in range(KT // WQ):
                        if keep < 0:
                            w_t = wp.tile([P, WQ, JG * P], BF16, name="w_t")
                            nc.sync.dma_start(
                                w_t[:, :, :nj * P],
                                wt_re[:, kq * WQ:(kq + 1) * WQ, js])
                        elif not deferred:
                            w_t = wg_sb[keep][:, kq * WQ:(kq + 1) * WQ, :]
                            nc.sync.dma_start(
                                w_t, wt_re[:, kq * WQ:(kq + 1) * WQ, js])
                        else:
                            w_t = wg_sb[keep][:, kq * WQ:(kq + 1) * WQ, :]
                        if kq == 1:
                            load_b(pi + 1)
                        for kw in range(WQ):
                            kt = kq * WQ + kw
                            for j in range(nj):
                                lhs = w_t[:, kw, j * P:(j + 1) * P]
                                for c in chunks:
                                    nc.tensor.matmul(
                                        psums[j, c], lhs,
                                        xt_sb[:, 4 * c:4 * (c + 1), kt],
                                        start=(kt == 0), stop=False)
                            if pi == 1 and kt in hooks:
                                hooks[kt]()
                    bt = b_tiles.pop(pi)
                    for j in range(nj):
                        for c in chunks:
                            cs = slice(c * NCH, (c + 1) * NCH)
                            nc.tensor.matmul(
                                psums[j, c], bt[:, 0, j * P:(j + 1) * P],
                                hta_sb[:, cs], start=False, stop=False)
                            nc.tensor.matmul(
                                psums[j, c], bt[:, 1, j * P:(j + 1) * P],
                                htb_sb[:, cs], start=False, stop=True)
                            ot = op_.tile([P, NCH], BF16, name="ot")
                            nc.vector.tensor_copy(ot[:], psums[j, c])
                            nc.scalar.dma_start(
                                out_d[(j0 + j) * P:(j0 + j + 1) * P, cs],
                                ot[:])
    nc.compile()
    return nc


_NC_CACHE = None
_LAST_IN_MAPS = None


def _get_nc():
    global _NC_CACHE
    if _NC_CACHE is None:
        _NC_CACHE = build_nc()
    return _NC_CACHE


def kernel(x, base_W, gate_W, lora_A, lora_B):
    x = np.asarray(x, dtype=np.float32)
    base_W = np.asarray(base_W, dtype=np.float32)
    gate_W = np.asarray(gate_W, dtype=np.float32)
    lora_A = np.asarray(lora_A, dtype=np.float32)
    lora_B = np.asarray(lora_B, dtype=np.float32)

    xf = x.reshape(B * S, D_IN)
    wt_np = np.ascontiguousarray(base_W.T).astype(NPBF16)     # [D_in, D_out]
    # ga packed [P, KT*256]: cols 0..223 = A_flat^T, 224..251 = gate_W^T
    big = np.zeros((D_IN, AT_COLS), dtype=np.float32)
    big[:, :ER] = lora_A.reshape(ER, D_IN).T
    big[:, ER:ER + N_EXPERTS] = gate_W.T
    ga_np = np.ascontiguousarray(
        big.reshape(KT, P, AT_COLS).transpose(1, 0, 2).reshape(
            P, KT * AT_COLS)).astype(NPBF16)
    # lora_B [E, D_out, R] -> b_flat [(e r), D_out]; rows 0..127 | 128..223
    b_flat = np.ascontiguousarray(
        lora_B.transpose(0, 2, 1).reshape(ER, D_OUT))
    bab_np = np.zeros((2 * P, D_OUT), dtype=np.float32)
    bab_np[:P] = b_flat[:P]
    bab_np[P:P + (ER - P)] = b_flat[P:]
    bab_np = bab_np.astype(NPBF16)

    in_maps = []
    for c in range(N_CORES):
        xc = xf[c * T:(c + 1) * T]                            # [T, D_in]
        # xg [TT, P, KT*128]: xg[t, p, kt*128+tok] = xc[t*128+tok, kt*128+p]
        xg_np = np.ascontiguousarray(
            xc.reshape(TT, P, KT, P).transpose(0, 3, 2, 1).reshape(
                TT, P, KT * P)).astype(NPBF16)
        in_maps.append({
            "xg": xg_np,
            "wt": wt_np,
            "ga": ga_np,
            "bab": bab_np,
        })

    global _LAST_IN_MAPS
    _LAST_IN_MAPS = in_maps
    nc = _get_nc()
    res = bass_utils.run_bass_kernel_spmd(nc, in_maps,
                                          core_ids=list(range(N_CORES)))
    out = np.empty((B * S, D_OUT), dtype=np.float32)
    for c in range(N_CORES):
        out[c * T:(c + 1) * T] = res.results[c]["out"].astype(np.float32).T
    return out.reshape(B, S, D_OUT)


# revision 13
# speedup vs baseline: 8.7976x; 8.7976x over previous
"""MoE LoRA linear kernel for Trainium2, 8 NeuronCores, data-parallel over tokens.

Reference computation (per token x, D=4096, E=28 experts, rank 8, top-2):
  base   = x @ W^T
  logits = x @ gate_W^T ; top-2 softmax -> per-expert gates g (0 elsewhere)
  h_e    = x @ A_e^T                     (all experts, rank 8)
  out    = base + sum_e g_e*2 * h_e @ B_e^T

Sharding: tokens split 8 ways (1024 tokens/core); weights replicated.

Numerics: everything bf16 on the PE (fp32 PSUM accumulation); inputs cast to
bf16 on host.  bf16 keeps the PE at 1 cycle/row and enables FWL (fast weight
load), which fp32/fp32r weights cannot use, halving the LDWEIGHTS cost of the
1024+ stationary loads in the base GEMM.  Output stored bf16, upcast on host.
End-to-end rel err ~2e-3 vs the 2e-2 gate.

Structure (per core):
  - x DMAs land directly in the resident x tile (no DVE cast): layout
    [P, TT, KT, 128] so each token tile is one contiguous 2MB DMA on the
    scalar-engine HWDGE queue (W uses the sync-engine queue).
  - phase-1 per token tile: 32 matmuls vs the combined [A^T | gate_W^T]
    (224 lora cols + 28 gate cols in one 256-wide PSUM accumulation), PSUM
    copied to bf16 h_all + f32 logits; batched top-2 softmax; gate-scale
    multiply; PE transpose -> h'T halves (hta/htb).
  - main GEMM in 13 passes over j-groups of 3 (2 PSUM banks x 512-token
    chunk): passes 0-1 run token-chunk 0 only and park their W tiles in SBUF
    (wg_sb); passes 2-10 run both chunks streaming W in 4-ktile chunks;
    passes 11-12 run the deferred chunk 1 of j 0-5 from the parked W tiles.
    Token tiles 4-7 phase-1 work is hooked inside pass 1's k-loop.
  - each pass ends with the 2 lora rank-combine matmuls per (j, chunk)
    accumulated into the same PSUM, bf16 copy-out, store.
"""
import sys

for _p in ("/opt/trn_rl_repo", "/root/.axon_site/_ro/trn_rl_repo"):
    if _p not in sys.path:
        sys.path.insert(0, _p)

import numpy as np
import ml_dtypes

import concourse.bass as bass
import concourse.mybir as mybir
import concourse.tile as tile
from concourse import bacc, bass_utils

F32 = mybir.dt.float32
BF16 = mybir.dt.bfloat16
NPBF16 = ml_dtypes.bfloat16

N_CORES = 8
B, S, D_IN, D_OUT = 4, 2048, 4096, 4096
N_EXPERTS, RANK, SCALING = 28, 8, 2.0
ER = N_EXPERTS * RANK          # 224
T = B * S // N_CORES           # 1024 tokens per core
P = 128
KT = D_IN // P                 # 32 k-tiles
JT = D_OUT // P                # 32 output row-tiles
TT = T // P                    # 8 token tiles
NCH = 512                      # moving free dim chunk (1 PSUM bank fp32)
JG = 3                         # j-tiles per psum group
WQ = 8                         # k-tiles per W DMA
AT_COLS = 256                  # 224 lora + 28 gate + 4 pad
NKEEP = 2                      # leading j-groups whose W stays resident


def build_nc(repeat=1):
    nc = bacc.Bacc("TRN2", target_bir_lowering=False, debug=False)
    xg_d = nc.dram_tensor("xg", [TT, P, KT * P], BF16,
                          kind="ExternalInput").ap()
    wt_d = nc.dram_tensor("wt", [D_IN, D_OUT], BF16, kind="ExternalInput").ap()
    ga_d = nc.dram_tensor("ga", [P, KT * AT_COLS], BF16,
                          kind="ExternalInput").ap()
    bab_d = nc.dram_tensor("bab", [2 * P, D_OUT], BF16,
                           kind="ExternalInput").ap()
    out_d = nc.dram_tensor("out", [D_OUT, T], BF16, kind="ExternalOutput").ap()

    ga_re = ga_d.rearrange("p (kt c) -> p kt c", kt=KT)
    wt_re = wt_d.rearrange("(kt p) j -> p kt j", p=P)
    bab_re = bab_d.rearrange("(i p) j -> p i j", p=P)

    # passes: (j0, nj, chunks, keep_idx)
    #   keep_idx >= 0 on the first NKEEP passes (W parked in wg_sb) and on
    #   their deferred chunk-1 passes at the end (W read back, no DMA).
    passes = [(g * JG, JG, (0,), g) for g in range(NKEEP)]
    ns = NKEEP * JG
    nfull = (JT - ns) // JG
    passes += [(ns + g * JG, JG, (0, 1), -1) for g in range(nfull)]
    if (JT - ns) % JG:
        passes.append((ns + nfull * JG, (JT - ns) % JG, (0, 1), -1))
    passes += [(g * JG, JG, (1,), g) for g in range(NKEEP)]

    with tile.TileContext(nc) as tc:
        with (
            tc.tile_pool(name="resident", bufs=1) as rp,
            tc.tile_pool(name="wstream", bufs=4) as wp,
            tc.tile_pool(name="bstream", bufs=2) as bp,
            tc.tile_pool(name="outstage", bufs=4) as op_,
            tc.tile_pool(name="smalls", bufs=2) as sp,
            tc.tile_pool(name="gating", bufs=1) as gp,
            tc.tile_pool(name="ph1ps", bufs=2, space="PSUM") as ph1,
            tc.tile_pool(name="psmm", bufs=6, space="PSUM") as psm,
        ):
            for _rep in range(repeat):
                ga_sb = rp.tile([P, KT, AT_COLS], BF16)
                nc.sync.dma_start(ga_sb[:], ga_re[:])
                xt_sb = rp.tile([P, TT, KT, P], BF16)
                wg_sb = [rp.tile([P, KT, JG * P], BF16, name=f"wg{g}")
                         for g in range(NKEEP)]
                h_all = rp.tile([P, TT, AT_COLS], BF16)
                hta_sb = rp.tile([P, T], BF16)
                htb_sb = rp.tile([P, T], BF16)
                logits_all = rp.tile([P, TT, N_EXPERTS], F32)
                gsc_all = rp.tile([P, TT, N_EXPERTS], F32)

                for t in range(TT):
                    # two half-DMAs so tile_mms can start on kt 0-15 while
                    # kt 16-31 still streams
                    xg_re = xg_d[t].rearrange("p (kt f) -> p kt f", kt=KT)
                    nc.scalar.dma_start(xt_sb[:, t, :KT // 2], xg_re[:, :KT // 2])
                    nc.scalar.dma_start(xt_sb[:, t, KT // 2:], xg_re[:, KT // 2:])

                def tile_mms(t):
                    """32 matmuls vs [A^T | gate_W^T]; stash h (bf16) and
                    logits (f32)."""
                    ph = ph1.tile([P, AT_COLS], F32, name="ph", tag="ph1")
                    for kt in range(KT):
                        nc.tensor.matmul(ph[:], xt_sb[:, t, kt], ga_sb[:, kt],
                                         start=(kt == 0), stop=(kt == KT - 1))
                    nc.vector.tensor_copy(logits_all[:, t],
                                          ph[:, ER:ER + N_EXPERTS])
                    nc.vector.tensor_copy(h_all[:, t], ph[:])

                def gate_chain(lo, hi):
                    """Batched top-2 softmax (x SCALING) for tiles [lo, hi)."""
                    n = hi - lo
                    sl = slice(lo, hi)
                    EB = (P, n, N_EXPERTS)
                    m1 = gp.tile([P, n], F32, name=f"m1_{lo}", tag="m1")
                    nc.vector.reduce_max(m1[:], logits_all[:, sl],
                                         axis=mybir.AxisListType.X)
                    m1b = m1[:, :, None].to_broadcast(EB)
                    eq = gp.tile([P, n, N_EXPERTS], F32, name=f"eq_{lo}",
                                 tag="eq")
                    nc.vector.tensor_tensor(eq[:], logits_all[:, sl], m1b,
                                            mybir.AluOpType.is_equal)
                    nc.vector.scalar_tensor_tensor(
                        eq[:], eq[:], -1e30, logits_all[:, sl],
                        mybir.AluOpType.mult, mybir.AluOpType.add)
                    m2 = gp.tile([P, n], F32, name=f"m2_{lo}", tag="m2")
                    nc.vector.reduce_max(m2[:], eq[:],
                                         axis=mybir.AxisListType.X)
                    mask2 = gp.tile([P, n, N_EXPERTS], F32, name=f"mask2_{lo}",
                                    tag="mask2")
                    nc.vector.tensor_tensor(mask2[:], logits_all[:, sl],
                                            m2[:, :, None].to_broadcast(EB),
                                            mybir.AluOpType.is_ge)
                    d1 = gp.tile([P, n, N_EXPERTS], F32, name=f"d1_{lo}",
                                 tag="d1")
                    nc.vector.tensor_tensor(d1[:], logits_all[:, sl], m1b,
                                            mybir.AluOpType.subtract)
                    nc.scalar.activation(d1[:], d1[:],
                                         mybir.ActivationFunctionType.Exp)
                    d2 = gp.tile([P, n], F32, name=f"d2_{lo}", tag="d2")
                    nc.vector.tensor_tensor(d2[:], m2[:], m1[:],
                                            mybir.AluOpType.subtract)
                    nc.scalar.activation(d2[:], d2[:],
                                         mybir.ActivationFunctionType.Exp)
                    nc.vector.tensor_scalar_add(d2[:], d2[:], 1.0)
                    nc.vector.reciprocal(d2[:], d2[:])
                    nc.vector.tensor_scalar_mul(d2[:], d2[:], SCALING)
                    nc.vector.tensor_tensor(d1[:], d1[:], mask2[:],
                                            mybir.AluOpType.mult)
                    nc.vector.tensor_tensor(gsc_all[:, sl], d1[:],
                                            d2[:, :, None].to_broadcast(EB),
                                            mybir.AluOpType.mult)

                def scale_transpose(t):
                    """h * gate-scale (bf16) then XBAR DMA-transpose into h'T
                    (keeps the transposes off the PE instruction stream)."""
                    ts_ = slice(t * P, (t + 1) * P)
                    hp = sp.tile([P, AT_COLS], BF16, name="hp")
                    nc.vector.memset(hp[:, ER:], 0.0)
                    nc.vector.tensor_tensor(
                        hp[:, :ER].rearrange("p (e r) -> p e r", r=RANK),
                        h_all[:, t, :ER].rearrange("p (e r) -> p e r", r=RANK),
                        gsc_all[:, t, :, None].to_broadcast(
                            (P, N_EXPERTS, RANK)),
                        mybir.AluOpType.mult)
                    for half, dst in ((0, hta_sb), (1, htb_sb)):
                        nc.scalar.dma_start(
                            dst[:, ts_], hp[:, half * P:(half + 1) * P],
                            transpose=True)

                # All phase-1 work is hooked into the k-loops of passes 0-2,
                # so the base GEMM starts as soon as the first x half-tiles
                # and W chunk land (no serial prologue).  Pass 0 (chunk 0)
                # needs hta/htb[:, 0:512] only for its final B-combine, so
                # tiles 0-3 run inside pass 0 itself.
                hooks = {
                    0: {
                        2: lambda: tile_mms(0),
                        8: lambda: tile_mms(1),
                        14: lambda: tile_mms(2),
                        19: lambda: tile_mms(3),
                        23: lambda: gate_chain(0, TT // 2),
                        25: lambda: scale_transpose(0),
                        27: lambda: scale_transpose(1),
                        28: lambda: scale_transpose(2),
                        29: lambda: scale_transpose(3),
                    },
                    1: {
                        4: lambda: tile_mms(4),
                        12: lambda: tile_mms(5),
                        20: lambda: tile_mms(6),
                        26: lambda: tile_mms(7),
                    },
                    2: {
                        2: lambda: gate_chain(TT // 2, TT),
                        6: lambda: scale_transpose(4),
                        10: lambda: scale_transpose(5),
                        14: lambda: scale_transpose(6),
                        18: lambda: scale_transpose(7),
                    },
                }

                b_tiles = {}

                def load_b(pi):
                    if pi >= len(passes):
                        return
                    j0, nj, _, _ = passes[pi]
                    js = slice(j0 * P, (j0 + nj) * P)
                    bt = bp.tile([P, 2, JG * P], BF16, name="bt")
                    nc.sync.dma_start(bt[:, :, :nj * P], bab_re[:, :, js])
                    b_tiles[pi] = bt

                load_b(0)
                for pi, (j0, nj, chunks, keep) in enumerate(passes):
                    js = slice(j0 * P, (j0 + nj) * P)
                    deferred = keep >= 0 and pi >= len(passes) - NKEEP
                    psums = {
                        (j, c): psm.tile([P, NCH], F32, name=f"pm_{j}_{c}",
                                         tag="pm")
                        for j in range(nj) for c in chunks
                    }
                    for kq in range(KT // WQ):
                        if keep < 0:
                            w_t = wp.tile([P, WQ, JG * P], BF16, name="w_t")
                            nc.sync.dma_start(
                                w_t[:, :, :nj * P],
                                wt_re[:, kq * WQ:(kq + 1) * WQ, js])
                        elif not deferred:
                            w_t = wg_sb[keep][:, kq * WQ:(kq + 1) * WQ, :]
                            nc.sync.dma_start(
                                w_t, wt_re[:, kq * WQ:(kq + 1) * WQ, js])
                        else:
                            w_t = wg_sb[keep][:, kq * WQ:(kq + 1) * WQ, :]
                        if kq == 1:
                            load_b(pi + 1)
                        for kw in range(WQ):
                            kt = kq * WQ + kw
                            for j in range(nj):
                                lhs = w_t[:, kw, j * P:(j + 1) * P]
                                for c in chunks:
                                    nc.tensor.matmul(
                                        psums[j, c], lhs,
                                        xt_sb[:, 4 * c:4 * (c + 1), kt],
                                        start=(kt == 0), stop=False)
                            if pi in hooks and kt in hooks[pi]:
                                hooks[pi][kt]()
                    bt = b_tiles.pop(pi)
                    for j in range(nj):
                        for c in chunks:
                            cs = slice(c * NCH, (c + 1) * NCH)
                            nc.tensor.matmul(
                                psums[j, c], bt[:, 0, j * P:(j + 1) * P],
                                hta_sb[:, cs], start=False, stop=False)
                            nc.tensor.matmul(
                                psums[j, c], bt[:, 1, j * P:(j + 1) * P],
                                htb_sb[:, cs], start=False, stop=True)
                            ot = op_.tile([P, NCH], BF16, name="ot")
                            # Activation-engine copy keeps PSUM bank frees
                            # off the DVE queue (busy with gating math).
                            nc.scalar.activation(
                                ot[:], psums[j, c],
                                mybir.ActivationFunctionType.Copy)
                            nc.sync.dma_start(
                                out_d[(j0 + j) * P:(j0 + j + 1) * P, cs],
                                ot[:])
    nc.compile()
    return nc


_NC_CACHE = None
_LAST_IN_MAPS = None


def _get_nc():
    global _NC_CACHE
    if _NC_CACHE is None:
        _NC_CACHE = build_nc()
    return _NC_CACHE


def kernel(x, base_W, gate_W, lora_A, lora_B):
    x = np.asarray(x, dtype=np.float32)
    base_W = np.asarray(base_W, dtype=np.float32)
    gate_W = np.asarray(gate_W, dtype=np.float32)
    lora_A = np.asarray(lora_A, dtype=np.float32)
    lora_B = np.asarray(lora_B, dtype=np.float32)

    xf = x.reshape(B * S, D_IN)
    wt_np = np.ascontiguousarray(base_W.T).astype(NPBF16)     # [D_in, D_out]
    # ga packed [P, KT*256]: cols 0..223 = A_flat^T, 224..251 = gate_W^T
    big = np.zeros((D_IN, AT_COLS), dtype=np.float32)
    big[:, :ER] = lora_A.reshape(ER, D_IN).T
    big[:, ER:ER + N_EXPERTS] = gate_W.T
    ga_np = np.ascontiguousarray(
        big.reshape(KT, P, AT_COLS).transpose(1, 0, 2).reshape(
            P, KT * AT_COLS)).astype(NPBF16)
    # lora_B [E, D_out, R] -> b_flat [(e r), D_out]; rows 0..127 | 128..223
    b_flat = np.ascontiguousarray(
        lora_B.transpose(0, 2, 1).reshape(ER, D_OUT))
    bab_np = np.zeros((2 * P, D_OUT), dtype=np.float32)
    bab_np[:P] = b_flat[:P]
    bab_np[P:P + (ER - P)] = b_flat[P:]
    bab_np = bab_np.astype(NPBF16)

    in_maps = []
    for c in range(N_CORES):
        xc = xf[c * T:(c + 1) * T]                            # [T, D_in]
        # xg [TT, P, KT*128]: xg[t, p, kt*128+tok] = xc[t*128+tok, kt*128+p]
        xg_np = np.ascontiguousarray(
            xc.reshape(TT, P, KT, P).transpose(0, 3, 2, 1).reshape(
                TT, P, KT * P)).astype(NPBF16)
        in_maps.append({
            "xg": xg_np,
            "wt": wt_np,
            "ga": ga_np,
            "bab": bab_np,
        })

    global _LAST_IN_MAPS
    _LAST_IN_MAPS = in_maps
    nc = _get_nc()
    res = bass_utils.run_bass_kernel_spmd(nc, in_maps,
                                          core_ids=list(range(N_CORES)))
    out = np.empty((B * S, D_OUT), dtype=np.float32)
    for c in range(N_CORES):
        out[c * T:(c + 1) * T] = res.results[c]["out"].astype(np.float32).T
    return out.reshape(B, S, D_OUT)
